# revision 8
# baseline (speedup 1.0000x reference)
"""DotPredictor on 8 Trainium2 NeuronCores.

score[e] = <h[src[e]], h[dst[e]]> ; edge_index [2, 600000] int64, h [100000,128] f32.

Strategy (data-parallel over edges, h replicated per core):
  - 75,000 edges per core.
  - h is converted to bf16 on the host (rows become 256 B — the dma_gather
    minimum — and the dot-product tolerance is ~4e-3 rel, well under the
    2e-2 gate). Halves both HBM gather traffic and DVE element count
    vs f32.
  - Rows of h are gathered with the custom SWDGE `dma_gather` instruction
    (int16 indices). Since 100k rows exceed int16, node rows are split into
    4 buckets of 25,000 rows; each core's edges are grouped on the host by
    (src_bucket, dst_bucket) -> 16 groups, each gathered against the right
    h base offset with bucket-local indices. Within a group, edges are
    sorted by src so the src gathers walk ascending addresses (DRAM row
    locality); dst stays random within the 6.4 MB bucket.
  - Gathers are issued in 1024-index chunks, round-robin across the 4
    SWDGE queues (the runtime's descriptor ring caps at ~65 descs/engine
    per queue; 2048-idx chunks fail on HW even with a larger
    dynamic_dma_scratch_size). The binding constraint is the GPSIMD
    engine: gathers execute serially on the one POOL sequencer and the
    Q7 ucode spends ~2.3 cycles/index (measured ~1.9 ns/row,
    insensitive to idx content and row size), so the gather pipeline
    runs at ~150k rows / ~280 us per core regardless of HBM locality.
  - Each chunk pair (src rows, dst rows; [128, K/128, 128] bf16 tiles, edge
    (tile-col, partition) layout) is multiplied and row-reduced on DVE into
    an f32 scores tile, stored contiguously at the end.
  - Host maps scores back through the per-core group sort permutation.

Group sizes vary per core; since the program is shared (SPMD), chunks are
padded to the static capacity with a ramp of distinct valid row indices
(descriptor counts must match the static num_idxs_reg -- a mismatch drifts
the SWDGE ring bookkeeping; and a constant pad row would hammer one DRAM
row). Padded lanes are discarded by the host-side inverse mapping.
"""

import ml_dtypes
import numpy as np

import concourse.bacc as bacc
import concourse.mybir as mybir
import concourse.tile as tile

N_CORES = 8
N_NODES = 100000
N_EDGES = 600000
D = 128
P = 128

E_PER_CORE = N_EDGES // N_CORES      # 75000
N_BUCKETS = 4
BUCKET = 25000                        # int16-safe local indices
N_GROUPS = N_BUCKETS * N_BUCKETS      # 16
K_GATHER = 1024                       # idxs per dma_gather (ring cap: 2048 fails)
N_QUEUES = 4
GATH_BUFS = 12                        # SBUF slots for gather tiles

H_DT = mybir.dt.bfloat16
H_NP = ml_dtypes.bfloat16


def plan(all_src, all_dst, k_gather=None):
    """Compute the shared static layout from the actual inputs.

    Returns (chunks_per_group [16], per-core group order/permutations).
    """
    k = k_gather or K_GATHER
    per_core = []
    max_group = np.zeros(N_GROUPS, dtype=np.int64)
    for c in range(N_CORES):
        lo = c * E_PER_CORE
        src = all_src[lo:lo + E_PER_CORE]
        dst = all_dst[lo:lo + E_PER_CORE]
        gid = (src // BUCKET) * N_BUCKETS + dst // BUCKET
        # group-major, src-ascending within group (DRAM row locality)
        order = np.lexsort((src, gid))
        sizes = np.bincount(gid, minlength=N_GROUPS)
        max_group = np.maximum(max_group, sizes)
        per_core.append((src, dst, order, sizes))
    chunk_sizes = []
    for g in range(N_GROUPS):
        mg = int(max_group[g])
        full = mg // k
        tail = mg - full * k
        sizes_g = [k] * full
        # tail rounded to 128-idx granularity (>=128) to cut pad traffic
        sizes_g.append(max(P, -(-tail // P) * P))
        chunk_sizes.append(sizes_g)
    return chunk_sizes, per_core


def build_program(chunks_per_group, repeats=1, do_dve=True, do_gather=True,
                  k_gather=None, scratch=None):
    k = k_gather or K_GATHER
    if scratch is None:
        # SWDGE ring capacity scales with the scratch carveout
        # (carveout_ndesc = scratch // 16); K idxs need K/16 + 1
        # descs/engine and we want 2 chunks in flight per queue.
        scratch = max(16384, 16 * k)
    chunk_tiles = k // P
    gath_bufs = max(6, min(GATH_BUFS, (4 * 2048) // k * 2))
    all_chunks = [kk for g in chunks_per_group for kk in g]
    g_tot = sum(kk // P for kk in all_chunks)      # scores columns
    idx_cols = 2 * sum(kk // 16 for kk in all_chunks)

    kw = {}
    if scratch:
        kw["dynamic_dma_scratch_size"] = scratch
    nc = bacc.Bacc("TRN2", target_bir_lowering=False, debug=False,
                   num_swdge_queues=N_QUEUES, **kw)
    h = nc.dram_tensor("h", [N_NODES, D], H_DT, kind="ExternalInput")
    idx = nc.dram_tensor("idx", [P, idx_cols], mybir.dt.int16,
                         kind="ExternalInput")
    out = nc.dram_tensor("scores", [P, g_tot], mybir.dt.float32,
                         kind="ExternalOutput")

    with tile.TileContext(nc) as tc:
        with (
            tc.tile_pool(name="idxp", bufs=1) as idx_pool,
            tc.tile_pool(name="sc", bufs=1) as sc_pool,
            tc.tile_pool(name="gp", bufs=gath_bufs) as gpool,
        ):
            idx_t = idx_pool.tile([P, idx_cols], mybir.dt.int16)
            # split the idx load so the first gathers start after ~1/16 of
            # the transfer instead of the whole 2.5 MB (tile deps are
            # view-range based)
            head = max(256, idx_cols // 16)
            nc.sync.dma_start(out=idx_t[:, :head], in_=idx[:, :head])
            nc.sync.dma_start(out=idx_t[:, head:], in_=idx[:, head:])
            scores = sc_pool.tile([P, g_tot], mybir.dt.float32)
            if not do_dve:
                nc.vector.memset(scores[:], 0.0)

            for _rep in range(repeats):
              gath_i, col0, idx0 = 0, 0, 0
              for g in range(N_GROUPS):
                bs, bd = divmod(g, N_BUCKETS)
                h_src = h[bs * BUCKET:(bs + 1) * BUCKET, :]
                h_dst = h[bd * BUCKET:(bd + 1) * BUCKET, :]
                for kk in chunks_per_group[g]:
                    ct = kk // P                    # tiles this chunk
                    S = kk // 16                    # idx cols this chunk
                    sidx0 = idx0
                    didx0 = idx0 + S
                    s_full = gpool.tile([P, chunk_tiles, D], H_DT, tag="s")
                    d_full = gpool.tile([P, chunk_tiles, D], H_DT, tag="d")
                    s_t = s_full[:, :ct, :]
                    d_t = d_full[:, :ct, :]
                    if do_gather:
                        nc.gpsimd.dma_gather(
                            out_ap=s_t[:], in_ap=h_src,
                            idxs_ap=idx_t[:, sidx0:sidx0 + S],
                            num_idxs=kk, num_idxs_reg=kk,
                            elem_size=D, queue_num=(2 * gath_i) % N_QUEUES)
                        nc.gpsimd.dma_gather(
                            out_ap=d_t[:], in_ap=h_dst,
                            idxs_ap=idx_t[:, didx0:didx0 + S],
                            num_idxs=kk, num_idxs_reg=kk,
                            elem_size=D, queue_num=(2 * gath_i + 1) % N_QUEUES)
                    if not do_gather:
                        nc.gpsimd.memset(s_t[:], 0.0)
                        nc.gpsimd.memset(d_t[:], 0.0)
                    if do_dve:
                        nc.vector.tensor_mul(out=s_t[:], in0=s_t[:], in1=d_t[:])
                        nc.vector.tensor_reduce(
                            out=scores[:, col0:col0 + ct],
                            in_=s_t[:],
                            axis=mybir.AxisListType.X,
                            op=mybir.AluOpType.add)
                    gath_i += 1
                    col0 += ct
                    idx0 += 2 * S

            nc.sync.dma_start(out=out[:], in_=scores[:])
    nc.compile()
    return nc


def _wrap_block(flat_i16):
    """[k] int16 -> [128, k/16] wrapped (idx j at [j%16, j//16]), replicated
    across the 8 groups of 16 partitions (each SWDGE queue pair reads its
    own)."""
    k = flat_i16.shape[0]
    w = flat_i16.reshape(k // 16, 16).T               # [16, k/16]
    return np.ascontiguousarray(np.tile(w, (8, 1)))   # [128, k/16]


def make_core_inputs(src, dst, order, sizes, chunks_per_group, h,
                     pad_value=0):
    """Build one core's idx tensor + the (p, col) mapping for its edges.

    h must already be bf16 [N_NODES, D]."""
    cap = np.array([sum(g) for g in chunks_per_group])  # idx capacity/group
    total = int(cap.sum())
    src_s = src[order]
    dst_s = dst[order]

    # spread pad slots over distinct rows -- a constant pad row would
    # hammer one DRAM row/bank and stall the SDMA engines
    ramp = (np.arange(total, dtype=np.int64) * 997) % BUCKET
    src_flat = ramp.copy()
    dst_flat = ramp.copy()
    gstart_e = np.concatenate([[0], np.cumsum(sizes)])      # edges
    gstart_c = np.concatenate([[0], np.cumsum(cap)])        # idx slots
    tile_start = np.concatenate(
        [[0], np.cumsum([sum(kk // P for kk in g) for g in chunks_per_group])])

    p_arr = np.empty(E_PER_CORE, dtype=np.int64)
    col_arr = np.empty(E_PER_CORE, dtype=np.int64)
    for g in range(N_GROUPS):
        n_g = int(sizes[g])
        e0, c0 = int(gstart_e[g]), int(gstart_c[g])
        bs, bd = divmod(g, N_BUCKETS)
        src_flat[c0:c0 + n_g] = src_s[e0:e0 + n_g] - bs * BUCKET
        dst_flat[c0:c0 + n_g] = dst_s[e0:e0 + n_g] - bd * BUCKET
        j = np.arange(n_g)
        p_arr[e0:e0 + n_g] = j % P
        col_arr[e0:e0 + n_g] = tile_start[g] + j // P

    # interleave [src_chunk, dst_chunk] wrapped segments per gather
    segs = []
    off = 0
    for g in chunks_per_group:
        for kk in g:
            segs.append(_wrap_block(src_flat[off:off + kk].astype(np.int16)))
            segs.append(_wrap_block(dst_flat[off:off + kk].astype(np.int16)))
            off += kk
    idx_np = np.concatenate(segs, axis=1)
    return ({"h": h, "idx": np.ascontiguousarray(idx_np)},
            (order, p_arr, col_arr))


def run(edge_index, h, pad_value=0):
    from concourse.bass_utils import run_bass_kernel_spmd

    h = np.asarray(h, dtype=np.float32).astype(H_NP)
    all_src = np.asarray(edge_index[0], dtype=np.int64)
    all_dst = np.asarray(edge_index[1], dtype=np.int64)
    chunks_per_group, per_core = plan(all_src, all_dst)
    nc = build_program(chunks_per_group)

    in_maps, mappings = [], []
    for c in range(N_CORES):
        src, dst, order, sizes = per_core[c]
        m, mapping = make_core_inputs(src, dst, order, sizes,
                                      chunks_per_group, h, pad_value)
        in_maps.append(m)
        mappings.append(mapping)

    res = run_bass_kernel_spmd(nc, in_maps, core_ids=list(range(N_CORES)))

    out = np.empty(N_EDGES, dtype=np.float32)
    for c in range(N_CORES):
        order, p_arr, col_arr = mappings[c]
        scores = res.results[c]["scores"]
        vals = scores[p_arr, col_arr]
        out[c * E_PER_CORE + order] = vals
    return out, res


def kernel(edge_index, h):
    out, _ = run(edge_index, h)
    return out


# revision 13
# speedup vs baseline: 1.1730x; 1.1730x over previous
"""DotPredictor on 8 Trainium2 NeuronCores.

score[e] = <h[src[e]], h[dst[e]]> ; edge_index [2, 600000] int64, h [100000,128] f32.

Strategy (data-parallel over edges, h replicated per core):
  - 75,000 edges per core.
  - h is converted to bf16 on the host (rows become 256 B — the dma_gather
    minimum — and the dot-product tolerance is ~4e-3 rel, well under the
    2e-2 gate). Halves both HBM gather traffic and DVE element count
    vs f32.
  - Rows of h are gathered with the custom SWDGE `dma_gather` instruction
    (int16 indices). Since 100k rows exceed int16, node rows are split into
    4 buckets of 25,000 rows; each core's edges are grouped on the host by
    (src_bucket, dst_bucket) -> 16 groups, each gathered against the right
    h base offset with bucket-local indices. Within a group, edges are
    sorted by src so the src gathers walk ascending addresses (DRAM row
    locality); dst stays random within the 6.4 MB bucket.
  - Gathers are issued in 1024-index chunks, round-robin across the 4
    SWDGE queues (the runtime's descriptor ring caps at ~65 descs/engine
    per queue; 2048-idx chunks fail on HW even with a larger
    dynamic_dma_scratch_size). The binding constraint is the GPSIMD
    engine: gathers execute serially on the one POOL sequencer and the
    Q7 ucode spends ~2.3 cycles/index (measured ~1.9 ns/row,
    insensitive to idx content and row size), so the gather pipeline
    runs at ~150k rows / ~280 us per core regardless of HBM locality.
  - Each chunk pair (src rows, dst rows; [128, K/128, 128] bf16 tiles, edge
    (tile-col, partition) layout) is multiplied and row-reduced on DVE into
    an f32 scores tile, stored contiguously at the end.
  - Host maps scores back through the per-core group sort permutation.

Group sizes vary per core; since the program is shared (SPMD), chunks are
padded to the static capacity with a ramp of distinct valid row indices
(descriptor counts must match the static num_idxs_reg -- a mismatch drifts
the SWDGE ring bookkeeping; and a constant pad row would hammer one DRAM
row). Padded lanes are discarded by the host-side inverse mapping.
"""

import ml_dtypes
import numpy as np

import concourse.bacc as bacc
import concourse.mybir as mybir
import concourse.tile as tile

N_CORES = 8
N_NODES = 100000
N_EDGES = 600000
D = 128
P = 128

E_PER_CORE = N_EDGES // N_CORES      # 75000
N_BUCKETS = 4
BUCKET = 25000                        # int16-safe local indices
N_GROUPS = N_BUCKETS * N_BUCKETS      # 16
K_GATHER = 1024                       # idxs per dma_gather (ring cap: 2048 fails)
N_QUEUES = 4
GATH_BUFS = 12                        # SBUF slots for gather tiles

H_DT = mybir.dt.bfloat16
H_NP = ml_dtypes.bfloat16


def plan(all_src, all_dst, k_gather=None):
    """Compute the shared static layout from the actual inputs.

    Edges are sorted globally by (group, src) and each group's sorted run
    is dealt to the 8 cores as contiguous sub-runs, so per-core group
    sizes are equal (+-1) and the SPMD capacity has ~zero padding (the Q7
    descriptor-generation wall is linear in static num_idxs, so every
    pad index costs ~1.9 ns).

    Returns (chunks_per_group [16], per-core (global_edge_ids, sizes)).
    """
    k = k_gather or K_GATHER
    gid = (all_src // BUCKET) * N_BUCKETS + all_dst // BUCKET
    order = np.lexsort((all_src, gid))     # group-major, src-ascending
    sizes_g = np.bincount(gid, minlength=N_GROUPS)
    per_core_segs = [[] for _ in range(N_CORES)]
    core_group_sizes = np.zeros((N_CORES, N_GROUPS), dtype=np.int64)
    chunk_sizes = []
    start = 0
    for g in range(N_GROUPS):
        run = order[start:start + int(sizes_g[g])]
        start += int(sizes_g[g])
        bounds = [round(i * len(run) / N_CORES) for i in range(N_CORES + 1)]
        for c in range(N_CORES):
            seg = run[bounds[c]:bounds[c + 1]]
            per_core_segs[c].append(seg)
            core_group_sizes[c, g] = len(seg)
        cap_g = int(core_group_sizes[:, g].max())
        full = cap_g // k
        tail = cap_g - full * k
        sizes_gc = [k] * full
        # tails at 16-idx granularity (idx wrap + ucode minimum)
        sizes_gc.append(max(16, -(-tail // 16) * 16))
        chunk_sizes.append(sizes_gc)
    per_core = [(np.concatenate(per_core_segs[c]), core_group_sizes[c])
                for c in range(N_CORES)]
    return chunk_sizes, per_core


def build_program(chunks_per_group, repeats=1, do_dve=True, do_gather=True,
                  k_gather=None, scratch=None):
    k = k_gather or K_GATHER
    if scratch is None:
        # SWDGE ring capacity scales with the scratch carveout
        # (carveout_ndesc = scratch // 16); K idxs need K/16 + 1
        # descs/engine and we want 2 chunks in flight per queue.
        scratch = max(16384, 16 * k)
    chunk_tiles = k // P
    gath_bufs = max(6, min(GATH_BUFS, (4 * 2048) // k * 2))
    all_chunks = [kk for g in chunks_per_group for kk in g]
    g_tot = sum(-(-kk // P) for kk in all_chunks)  # scores columns
    idx_cols = 2 * sum(kk // 16 for kk in all_chunks)

    kw = {}
    if scratch:
        kw["dynamic_dma_scratch_size"] = scratch
    nc = bacc.Bacc("TRN2", target_bir_lowering=False, debug=False,
                   num_swdge_queues=N_QUEUES, **kw)
    h = nc.dram_tensor("h", [N_NODES, D], H_DT, kind="ExternalInput")
    idx = nc.dram_tensor("idx", [P, idx_cols], mybir.dt.int16,
                         kind="ExternalInput")
    out = nc.dram_tensor("scores", [P, g_tot], mybir.dt.float32,
                         kind="ExternalOutput")

    with tile.TileContext(nc) as tc:
        with (
            tc.tile_pool(name="idxp", bufs=1) as idx_pool,
            tc.tile_pool(name="sc", bufs=1) as sc_pool,
            tc.tile_pool(name="gp", bufs=gath_bufs) as gpool,
        ):
            idx_t = idx_pool.tile([P, idx_cols], mybir.dt.int16)
            # split the idx load so the first gathers start after ~1/16 of
            # the transfer instead of the whole 2.5 MB (tile deps are
            # view-range based)
            head = max(256, idx_cols // 16)
            nc.sync.dma_start(out=idx_t[:, :head], in_=idx[:, :head])
            nc.sync.dma_start(out=idx_t[:, head:], in_=idx[:, head:])
            scores = sc_pool.tile([P, g_tot], mybir.dt.float32)
            if not do_dve:
                nc.vector.memset(scores[:], 0.0)

            for _rep in range(repeats):
              gath_i, col0, idx0 = 0, 0, 0
              for g in range(N_GROUPS):
                bs, bd = divmod(g, N_BUCKETS)
                h_src = h[bs * BUCKET:(bs + 1) * BUCKET, :]
                h_dst = h[bd * BUCKET:(bd + 1) * BUCKET, :]
                for kk in chunks_per_group[g]:
                    ct = -(-kk // P)                # tiles this chunk
                    S = kk // 16                    # idx cols this chunk
                    sidx0 = idx0
                    didx0 = idx0 + S
                    s_full = gpool.tile([P, chunk_tiles, D], H_DT, tag="s")
                    d_full = gpool.tile([P, chunk_tiles, D], H_DT, tag="d")
                    s_t = s_full[:, :ct, :]
                    d_t = d_full[:, :ct, :]
                    if do_gather:
                        nc.gpsimd.dma_gather(
                            out_ap=s_t[:], in_ap=h_src,
                            idxs_ap=idx_t[:, sidx0:sidx0 + S],
                            num_idxs=kk, num_idxs_reg=kk,
                            elem_size=D, queue_num=(2 * gath_i) % N_QUEUES)
                        nc.gpsimd.dma_gather(
                            out_ap=d_t[:], in_ap=h_dst,
                            idxs_ap=idx_t[:, didx0:didx0 + S],
                            num_idxs=kk, num_idxs_reg=kk,
                            elem_size=D, queue_num=(2 * gath_i + 1) % N_QUEUES)
                    if not do_gather:
                        nc.gpsimd.memset(s_t[:], 0.0)
                        nc.gpsimd.memset(d_t[:], 0.0)
                    if do_dve:
                        nc.vector.tensor_mul(out=s_t[:], in0=s_t[:], in1=d_t[:])
                        nc.vector.tensor_reduce(
                            out=scores[:, col0:col0 + ct],
                            in_=s_t[:],
                            axis=mybir.AxisListType.X,
                            op=mybir.AluOpType.add)
                    gath_i += 1
                    col0 += ct
                    idx0 += 2 * S

            nc.sync.dma_start(out=out[:], in_=scores[:])
    nc.compile()
    return nc


def _wrap_block(flat_i16):
    """[k] int16 -> [128, k/16] wrapped (idx j at [j%16, j//16]), replicated
    across the 8 groups of 16 partitions (each SWDGE queue pair reads its
    own)."""
    k = flat_i16.shape[0]
    w = flat_i16.reshape(k // 16, 16).T               # [16, k/16]
    return np.ascontiguousarray(np.tile(w, (8, 1)))   # [128, k/16]


def make_core_inputs(gids, sizes, chunks_per_group, all_src, all_dst, h,
                     pad_value=0):
    """Build one core's idx tensor + the (p, col) mapping for its edges.

    gids: this core's global edge ids, already group-major / src-sorted.
    h must already be bf16 [N_NODES, D]."""
    cap = np.array([sum(g) for g in chunks_per_group])  # idx capacity/group
    total = int(cap.sum())
    n_edges = len(gids)
    src_s = all_src[gids]
    dst_s = all_dst[gids]

    # spread pad slots over distinct rows -- a constant pad row would
    # hammer one DRAM row/bank and stall the SDMA engines
    ramp = (np.arange(total, dtype=np.int64) * 997) % BUCKET
    src_flat = ramp.copy()
    dst_flat = ramp.copy()
    gstart_e = np.concatenate([[0], np.cumsum(sizes)])      # edges
    gstart_c = np.concatenate([[0], np.cumsum(cap)])        # idx slots
    tile_start = np.concatenate(
        [[0], np.cumsum([sum(-(-kk // P) for kk in g)
                         for g in chunks_per_group])])

    p_arr = np.empty(n_edges, dtype=np.int64)
    col_arr = np.empty(n_edges, dtype=np.int64)
    for g in range(N_GROUPS):
        n_g = int(sizes[g])
        e0, c0 = int(gstart_e[g]), int(gstart_c[g])
        bs, bd = divmod(g, N_BUCKETS)
        src_flat[c0:c0 + n_g] = src_s[e0:e0 + n_g] - bs * BUCKET
        dst_flat[c0:c0 + n_g] = dst_s[e0:e0 + n_g] - bd * BUCKET
        j = np.arange(n_g)
        p_arr[e0:e0 + n_g] = j % P
        col_arr[e0:e0 + n_g] = tile_start[g] + j // P

    # interleave [src_chunk, dst_chunk] wrapped segments per gather
    segs = []
    off = 0
    for g in chunks_per_group:
        for kk in g:
            segs.append(_wrap_block(src_flat[off:off + kk].astype(np.int16)))
            segs.append(_wrap_block(dst_flat[off:off + kk].astype(np.int16)))
            off += kk
    idx_np = np.concatenate(segs, axis=1)
    return ({"h": h, "idx": np.ascontiguousarray(idx_np)},
            (gids, p_arr, col_arr))


def run(edge_index, h, pad_value=0):
    from concourse.bass_utils import run_bass_kernel_spmd

    h = np.asarray(h, dtype=np.float32).astype(H_NP)
    all_src = np.asarray(edge_index[0], dtype=np.int64)
    all_dst = np.asarray(edge_index[1], dtype=np.int64)
    chunks_per_group, per_core = plan(all_src, all_dst)
    nc = build_program(chunks_per_group)

    in_maps, mappings = [], []
    for c in range(N_CORES):
        gids, sizes = per_core[c]
        m, mapping = make_core_inputs(gids, sizes, chunks_per_group,
                                      all_src, all_dst, h, pad_value)
        in_maps.append(m)
        mappings.append(mapping)

    res = run_bass_kernel_spmd(nc, in_maps, core_ids=list(range(N_CORES)))

    out = np.empty(N_EDGES, dtype=np.float32)
    for c in range(N_CORES):
        gids, p_arr, col_arr = mappings[c]
        scores = res.results[c]["scores"]
        out[gids] = scores[p_arr, col_arr]
    return out, res


def kernel(edge_index, h):
    out, _ = run(edge_index, h)
    return out
